# revision 13
# baseline (speedup 1.0000x reference)
"""ECE loss kernel for Trainium2 (Bass/Tile), data-parallel over 8 NeuronCores.

Math (per sample row of logits[N, C]):
  conf = max softmax(x) = exp(max(x)) / sum(exp(x))
  acc  = (argmax(x) == label)
  ece  = sum_b |conf_sum[b] - acc_sum[b]| / N     (15 bins + empty tail)

Device pipeline per core (125184 rows as [128 partitions x 978 samples],
tiles of 16 samples x 100 classes):
  - DMA x tiles on the two HWDGE rings (sync/scalar)
  - ACT: E = exp(x) in place
  - Pool (GpSimd): two max tree-fold levels over E (100 -> 50 -> 25) —
    the otherwise-idle engine absorbs 3/4 of the row-max work; max-folds
    are numerically exact in any order
  - DVE: reduce_max over the 25-wide folds + reduce_sum over E (the only
    irreducible 1x pass), then conf/acc and 49 tensor_scalar+accum passes
    for the per-bin cumulative sums (wt/nn/av at 2x fp32) + total accuracy
Host: gathers g = x[i,label_i] (1% of bytes), pads 1472 zero rows, sums the
per-core [128, 49] outputs and finishes the tiny 16-bin reduction.
"""

import os

import numpy as np

import concourse.bass as bass
import concourse.mybir as mybir
import concourse.tile as tile
from concourse.bass_utils import run_bass_kernel_spmd

F32 = mybir.dt.float32
ALU = mybir.AluOpType
AX = mybir.AxisListType
ACTF = mybir.ActivationFunctionType

N = 1_000_000
C = 100
NCORES = 8
P = 128
SPP = 978                   # samples per partition (padded)
ROWS = P * SPP              # 125184 rows per core
NTOT = NCORES * ROWS        # 1001472
PAD = NTOT - N              # 1472 zero pad rows (conf=0.01, acc=1)
K = 32                      # samples per tile
SIZES = [K] * 30 + [18]     # sum = 978
NBINS = 16

LAST_RESULTS = None


def _bin_thresholds():
    """C_b = largest f32 y such that f32(15*y) <= b+1, for b = 0..14."""
    thr = []
    for b in range(15):
        tgt = np.float32(b + 1)

        def f(v):
            return np.float32(np.float32(15.0) * v)

        y = np.float32((b + 1) / 15.0)
        if f(y) <= tgt:
            while True:
                y2 = np.nextafter(y, np.float32(np.inf))
                if f(y2) <= tgt:
                    y = y2
                else:
                    break
        else:
            while f(y) > tgt:
                y = np.nextafter(y, np.float32(-np.inf))
        thr.append(np.float32(y))
    thr.append(np.float32(1e9))  # catch-all last segment
    return thr


THR = _bin_thresholds()


def _to_bf16(x):
    """Round f32 -> nearest-even bf16, returned as exactly-representable f32."""
    u = int(np.float32(x).view(np.uint32))
    upper, lower = u >> 16, u & 0xFFFF
    if lower > 0x8000 or (lower == 0x8000 and (upper & 1)):
        upper += 1
    return np.uint32(upper << 16).view(np.float32)


# bf16-representable bin boundaries: the device bins the bf16-rounded conf
# against these, and the host S_b identity uses the same exact values
THRB = [_to_bf16(t) for t in THR[:15]] + [np.float32(2.0 ** 30)]
PAD_CONF_BF = float(_to_bf16(np.float32(np.float32(1.0) * np.float32(1.0 / np.float32(100.0)))))


def _build():
    nc = bass.Bass(trn_type="TRN2")
    x = nc.dram_tensor("x", [P, SPP * C], F32, kind="ExternalInput")
    g = nc.dram_tensor("g", [P, SPP], F32, kind="ExternalInput")
    bins = nc.dram_tensor("bins", [P, 49], F32, kind="ExternalOutput")

    with tile.TileContext(nc) as tc:
        with (
            tc.tile_pool(name="xin", bufs=6) as xin,
            tc.tile_pool(name="persist", bufs=1) as persist,
        ):
            g_sb = persist.tile([P, SPP], F32)
            em = persist.tile([P, SPP], F32)
            s_t = persist.tile([P, SPP], F32)
            nc.gpsimd.dma_start(out=g_sb[:, :], in_=g[:, :])

            dma_engines = [nc.sync, nc.scalar]
            off = 0
            for t, k in enumerate(SIZES):
                kc = k * C
                xt = xin.tile([P, K * C], F32, tag="xt")
                dma_engines[t % 2].dma_start(
                    out=xt[:, :kc], in_=x[:, off * C : (off + k) * C]
                )
                nc.scalar.activation(xt[:, :kc], xt[:, :kc], ACTF.Exp)
                ev = xt[:, :kc].rearrange("p (k c) -> p k c", c=C)
                nc.vector.reduce_max(
                    out=em[:, off : off + k], in_=ev[:, :, :], axis=AX.X
                )
                nc.vector.reduce_sum(
                    out=s_t[:, off : off + k], in_=ev[:, :, :], axis=AX.X
                )
                off += k

            BF16 = mybir.dt.bfloat16
            rs = persist.tile([P, SPP], F32)
            conf = persist.tile([P, SPP], F32)
            conf_bf = persist.tile([P, SPP], BF16)
            acc_bf = persist.tile([P, SPP], BF16)
            junk_bf = persist.tile([P, SPP], BF16)
            bins_sb = persist.tile([P, 49], F32)

            nc.vector.reciprocal(rs[:, :], s_t[:, :])
            nc.vector.tensor_mul(conf[:, :], em[:, :], rs[:, :])
            nc.scalar.activation(g_sb[:, :], g_sb[:, :], ACTF.Exp)
            # acc = (exp(g) == em); bf16 out is exact for {0, 1}
            nc.vector.tensor_tensor(
                acc_bf[:, :], g_sb[:, :], em[:, :], op=ALU.is_equal
            )
            # bin on the bf16-rounded conf: all three per-bin sums see the
            # SAME rounded values, so per-sample bin assignment stays
            # consistent; the rounding only nudges boundary samples between
            # adjacent bins (net ECE effect ~1e-6)
            nc.vector.tensor_copy(conf_bf[:, :], conf[:, :])

            for b in range(NBINS):
                cb = float(THRB[b])
                nc.vector.tensor_scalar(
                    junk_bf[:, :], conf_bf[:, :], cb, None,
                    op0=ALU.min, op1=ALU.add,
                    accum_out=bins_sb[:, b : b + 1],
                )
                nc.vector.tensor_scalar(
                    junk_bf[:, :], conf_bf[:, :], cb, None,
                    op0=ALU.is_le, op1=ALU.add,
                    accum_out=bins_sb[:, 16 + b : 17 + b],
                )
                nc.vector.scalar_tensor_tensor(
                    junk_bf[:, :], conf_bf[:, :], cb, acc_bf[:, :],
                    op0=ALU.is_le, op1=ALU.mult,
                    accum_out=bins_sb[:, 32 + b : 33 + b],
                )
            nc.vector.tensor_scalar(
                junk_bf[:, :], acc_bf[:, :], 1.0, None,
                op0=ALU.mult, op1=ALU.add,
                accum_out=bins_sb[:, 48:49],
            )
            nc.sync.dma_start(out=bins[:, :], in_=bins_sb[:, :])

    # ---- sync-command budget fixes (instructions carry <= 2 sync commands:
    # completion update + at most one wait).  Drop waits provably covered by
    # earlier waits on the same engine, then split any remaining multi-wait
    # instruction across preceding same-engine drains.
    import re as _re

    def _tick_sem(name):
        return bool(_re.match(
            r"^(Activation|DVE|PE|Pool|SP|DMAHW\d+|DMASW\d+)_\d+$", name
        ))

    seen_waits = {}
    for bb in nc.m.functions[0].blocks:
        for ins in bb.instructions:
            si = ins.sync_info
            if si is None:
                continue
            tname = type(ins).__name__
            if tname == "InstEventSemaphore":
                continue
            eng = str(ins.engine).split(".")[-1]
            kept = list(si.on_wait)
            if tname not in ("InstDMACopy", "InstDrain") and len(kept) > 1:
                # same-engine waits are redundant (program order)
                kept = [w for w in kept if not w.ant_name.startswith(f"{eng}_")]
            kept2 = []
            for w in kept:
                if not _tick_sem(w.ant_name):
                    kept2.append(w)
                elif seen_waits.get((eng, w.ant_name), -1) < w.wait_value:
                    kept2.append(w)
            kept = kept2
            for w in kept:
                if not _tick_sem(w.ant_name):
                    continue
                key = (eng, w.ant_name)
                seen_waits[key] = max(seen_waits.get(key, -1), w.wait_value)
            if len(kept) != len(si.on_wait):
                si.on_wait = kept
                ins.sync_info = si

    import bass_rust as _br

    for bb in nc.m.functions[0].blocks:
        while True:
            insns = list(bb.instructions)
            target = None
            for idx, ins in enumerate(insns):
                si = ins.sync_info
                if si is None:
                    continue
                if len(si.on_wait) > 1:
                    target = (idx, ins)
                    break
            if target is None:
                break
            idx, ins = target
            si = ins.sync_info
            waits = list(si.on_wait)
            if type(ins).__name__ == "InstDrain":
                room = max(0, 1 - len(si.on_update))
            else:
                room = 1
            keep, extra = waits[len(waits) - room :], waits[: len(waits) - room]
            pos = idx
            for i, w in enumerate(extra):
                nd = mybir.InstDrain(
                    name=f"{ins.name}-presync{i}", ins=[], outs=[],
                    bass_is_fusable=False,
                )
                nd.engine = ins.engine
                nd.sync_info = _br.SyncInfo(on_wait=[w], on_update=[])
                nc.register_instruction(nd, overwrite=True)
                bb.instructions.insert(pos, nd)
                pos += 1
            si.on_wait = keep
            ins.sync_info = si
    return nc


_NC_CACHE = {}


def _get_nc():
    if "nc" not in _NC_CACHE:
        _NC_CACHE["nc"] = _build()
    return _NC_CACHE["nc"]


def kernel(logits, labels):
    global LAST_RESULTS
    logits = np.ascontiguousarray(np.asarray(logits), dtype=np.float32)
    labels_i = np.asarray(labels).astype(np.int64)
    assert logits.shape == (N, C), logits.shape

    # host-side gather of the label logit (1% of input bytes)
    gvals = logits[np.arange(N), labels_i].astype(np.float32)

    in_maps = []
    for c in range(NCORES):
        lo, hi = c * ROWS, (c + 1) * ROWS
        if hi <= N:
            xs = logits[lo:hi]
            gc = gvals[lo:hi]
        else:
            xs = np.concatenate(
                [logits[lo:], np.zeros((hi - N, C), np.float32)], axis=0
            )
            gc = np.concatenate([gvals[lo:], np.zeros(hi - N, np.float32)])
        in_maps.append(
            {
                "x": np.ascontiguousarray(xs.reshape(P, SPP * C)),
                "g": np.ascontiguousarray(gc.reshape(P, SPP)),
            }
        )

    trace = bool(int(os.environ.get("ECE_TRACE", "0")))
    res = run_bass_kernel_spmd(
        _get_nc(), in_maps, core_ids=list(range(NCORES)), trace=trace
    )
    LAST_RESULTS = res

    wt = np.zeros(NBINS, np.float64)
    nn_ = np.zeros(NBINS, np.float64)
    av = np.zeros(NBINS, np.float64)
    ac_tot = 0.0
    for out in res.results:
        ob = out["bins"].astype(np.float64)
        wt += ob[:, 0:16].sum(axis=0)
        nn_ += ob[:, 16:32].sum(axis=0)
        av += ob[:, 32:48].sum(axis=0)
        ac_tot += ob[:, 48].sum()

    # remove the PAD rows (conf_bf = bf16(0.01) <= every threshold, acc=1)
    wt -= PAD * PAD_CONF_BF
    nn_ -= PAD
    av -= PAD

    thr64 = np.array([np.float64(t) for t in THRB])
    S = wt - thr64 * (N - nn_)
    S[15] = wt[15]
    conf_sum = np.diff(S, prepend=0.0)
    acc_sum = np.diff(av, prepend=0.0)
    ece = np.abs(conf_sum - acc_sum).sum() / N
    return np.array([ece], dtype=np.float32)


# revision 23
# speedup vs baseline: 1.2478x; 1.2478x over previous
"""ECE loss kernel for Trainium2 (Bass/Tile), data-parallel over 8 NeuronCores.

Math (per sample row of logits[N, C]):
  conf = max softmax(x) = exp(max(x)) / sum(exp(x))
  acc  = (argmax(x) == label)
  ece  = sum_b |conf_sum[b] - acc_sum[b]| / N     (15 bins + empty tail)

Device pipeline per core (125184 rows as [128 partitions x 978 samples],
tiles of 16 samples x 100 classes):
  - DMA x tiles on the two HWDGE rings (sync/scalar)
  - ACT: E = exp(x) in place
  - Pool (GpSimd): two max tree-fold levels over E (100 -> 50 -> 25) —
    the otherwise-idle engine absorbs 3/4 of the row-max work; max-folds
    are numerically exact in any order
  - DVE: reduce_max over the 25-wide folds + reduce_sum over E (the only
    irreducible 1x pass), then conf/acc and 49 tensor_scalar+accum passes
    for the per-bin cumulative sums (wt/nn/av at 2x fp32) + total accuracy
Host: gathers g = x[i,label_i] (1% of bytes), pads 1472 zero rows, sums the
per-core [128, 49] outputs and finishes the tiny 16-bin reduction.
"""

import os

import numpy as np

import concourse.bass as bass
import concourse.mybir as mybir
import concourse.tile as tile
from concourse.bass_utils import run_bass_kernel_spmd

F32 = mybir.dt.float32
ALU = mybir.AluOpType
AX = mybir.AxisListType
ACTF = mybir.ActivationFunctionType

N = 1_000_000
C = 100
NCORES = 8
P = 128
SPP = 978                   # samples per partition (padded)
ROWS = P * SPP              # 125184 rows per core
NTOT = NCORES * ROWS        # 1001472
PAD = NTOT - N              # 1472 zero pad rows (conf=0.01, acc=1)
K = 32                      # samples per tile
SIZES = [K] * 30 + [18]     # sum = 978
CHUNKS = [(0, 384), (384, 768), (768, 978)]  # binning chunks (tile-aligned)
NBINS = 16

LAST_RESULTS = None


def _bin_thresholds():
    """C_b = largest f32 y such that f32(15*y) <= b+1, for b = 0..14."""
    thr = []
    for b in range(15):
        tgt = np.float32(b + 1)

        def f(v):
            return np.float32(np.float32(15.0) * v)

        y = np.float32((b + 1) / 15.0)
        if f(y) <= tgt:
            while True:
                y2 = np.nextafter(y, np.float32(np.inf))
                if f(y2) <= tgt:
                    y = y2
                else:
                    break
        else:
            while f(y) > tgt:
                y = np.nextafter(y, np.float32(-np.inf))
        thr.append(np.float32(y))
    thr.append(np.float32(1e9))  # catch-all last segment
    return thr


THR = _bin_thresholds()


def _to_bf16(x):
    """Round f32 -> nearest-even bf16, returned as exactly-representable f32."""
    u = int(np.float32(x).view(np.uint32))
    upper, lower = u >> 16, u & 0xFFFF
    if lower > 0x8000 or (lower == 0x8000 and (upper & 1)):
        upper += 1
    return np.uint32(upper << 16).view(np.float32)


# bf16-representable bin boundaries: the device bins the bf16-rounded conf
# against these, and the host S_b identity uses the same exact values
THRB = [_to_bf16(t) for t in THR[:15]] + [np.float32(2.0 ** 30)]
PAD_CONF_BF = float(_to_bf16(np.float32(np.float32(1.0) * np.float32(1.0 / np.float32(100.0)))))


def _next_bf16_up(x):
    u = int(np.float32(x).view(np.uint32))
    return np.uint32(((u >> 16) + 1) << 16).view(np.float32)


# off-grid thresholds strictly between consecutive bf16 values: a bf16 conf
# can never equal one, so sign(conf - c') is exactly +-1 and
# (N - sum sign)/2 counts {conf <= c_b} exactly
THRP = [
    np.float32(
        np.float64(THRB[b]) + (np.float64(_next_bf16_up(THRB[b])) - np.float64(THRB[b])) / 2
    )
    for b in range(15)
] + [np.float32(1.5 * 2.0 ** 30)]
# u = conf - 2*acc thresholds: {u <= c'-2} == {acc=1 and conf <= c_b}
THRU = [np.float32(np.float32(t) - np.float32(2.0)) for t in THRP[:15]] + [
    np.float32(-0.5)
]


def _build():
    nc = bass.Bass(trn_type="TRN2")
    x = nc.dram_tensor("x", [P, SPP * C], F32, kind="ExternalInput")
    g = nc.dram_tensor("g", [P, SPP], F32, kind="ExternalInput")
    cn = nc.dram_tensor("cn", [P, 32], F32, kind="ExternalInput")
    bins = nc.dram_tensor("bins", [P, 49 * len(CHUNKS)], F32, kind="ExternalOutput")

    with tile.TileContext(nc) as tc:
        BF16 = mybir.dt.bfloat16
        with (
            tc.tile_pool(name="xin", bufs=6) as xin,
            tc.tile_pool(name="persist", bufs=1) as persist,
        ):
            g_sb = persist.tile([P, SPP], F32)
            em = persist.tile([P, SPP], F32)
            s_t = persist.tile([P, SPP], F32)
            rs = persist.tile([P, SPP], F32)
            conf_bf = persist.tile([P, SPP], BF16)
            acc_bf = persist.tile([P, SPP], BF16)
            bins_sb = persist.tile([P, 49 * len(CHUNKS)], F32)
            cn_sb = persist.tile([P, 32], F32)
            nc.gpsimd.dma_start(out=g_sb[:, :], in_=g[:, :])
            nc.gpsimd.dma_start(out=cn_sb[:, :], in_=cn[:, :])

            def emit_chunk(ci, lo, hi):
                sl = slice(lo, hi)
                base = 49 * ci
                nc.vector.reciprocal(rs[:, sl], s_t[:, sl])
                # conf_bf = bf16(em / s): all three per-bin sums see the SAME
                # rounded values, so per-sample bin assignment is consistent;
                # rounding only nudges boundary samples between adjacent bins
                nc.vector.tensor_mul(conf_bf[:, sl], em[:, sl], rs[:, sl])
                nc.scalar.activation(g_sb[:, sl], g_sb[:, sl], ACTF.Exp)
                nc.vector.tensor_tensor(
                    acc_bf[:, sl], g_sb[:, sl], em[:, sl], op=ALU.is_equal
                )
                # u = conf - 2*acc  (reuses rs; exact in fp32)
                nc.vector.scalar_tensor_tensor(
                    rs[:, sl], acc_bf[:, sl], -2.0, conf_bf[:, sl],
                    op0=ALU.mult, op1=ALU.add,
                )
                junk = s_t  # dead after reciprocal
                nc.scalar.activation(
                    junk[:, sl], conf_bf[:, sl], ACTF.Copy,
                    accum_out=bins_sb[:, base + 48 : base + 49],
                )
                for b in range(NBINS):
                    nc.scalar.activation(
                        junk[:, sl], conf_bf[:, sl], ACTF.Relu,
                        bias=cn_sb[:, b : b + 1],
                        accum_out=bins_sb[:, base + b : base + b + 1],
                    )
                for b in range(NBINS):
                    nc.scalar.activation(
                        junk[:, sl], conf_bf[:, sl], ACTF.Sign,
                        bias=cn_sb[:, b : b + 1],
                        accum_out=bins_sb[:, base + 16 + b : base + 17 + b],
                    )
                for b in range(NBINS):
                    nc.scalar.activation(
                        junk[:, sl], rs[:, sl], ACTF.Sign,
                        bias=cn_sb[:, 16 + b : 17 + b],
                        accum_out=bins_sb[:, base + 32 + b : base + 33 + b],
                    )

            dma_engines = [nc.sync, nc.scalar]
            off = 0
            ci = 0
            for t, k in enumerate(SIZES):
                kc = k * C
                xt = xin.tile([P, K * C], F32, tag="xt")
                dma_engines[t % 2].dma_start(
                    out=xt[:, :kc], in_=x[:, off * C : (off + k) * C]
                )
                nc.scalar.activation(xt[:, :kc], xt[:, :kc], ACTF.Exp)
                ev = xt[:, :kc].rearrange("p (k c) -> p k c", c=C)
                nc.vector.reduce_max(
                    out=em[:, off : off + k], in_=ev[:, :, :], axis=AX.X
                )
                nc.vector.reduce_sum(
                    out=s_t[:, off : off + k], in_=ev[:, :, :], axis=AX.X
                )
                off += k
                if ci < len(CHUNKS) and CHUNKS[ci][1] == off:
                    emit_chunk(ci, CHUNKS[ci][0], CHUNKS[ci][1])
                    ci += 1
            assert ci == len(CHUNKS)
            nc.sync.dma_start(out=bins[:, :], in_=bins_sb[:, :])

    # ---- sync-command budget fixes (instructions carry <= 2 sync commands:
    # completion update + at most one wait).  Drop waits provably covered by
    # earlier waits on the same engine, then split any remaining multi-wait
    # instruction across preceding same-engine drains.
    import re as _re

    def _tick_sem(name):
        return bool(_re.match(
            r"^(Activation|DVE|PE|Pool|SP|DMAHW\d+|DMASW\d+)_\d+$", name
        ))

    seen_waits = {}
    for bb in nc.m.functions[0].blocks:
        for ins in bb.instructions:
            si = ins.sync_info
            if si is None:
                continue
            tname = type(ins).__name__
            if tname == "InstEventSemaphore":
                continue
            eng = str(ins.engine).split(".")[-1]
            kept = list(si.on_wait)
            if tname not in ("InstDMACopy", "InstDrain") and len(kept) > 1:
                # same-engine waits are redundant (program order)
                kept = [w for w in kept if not w.ant_name.startswith(f"{eng}_")]
            kept2 = []
            for w in kept:
                if not _tick_sem(w.ant_name):
                    kept2.append(w)
                elif seen_waits.get((eng, w.ant_name), -1) < w.wait_value:
                    kept2.append(w)
            kept = kept2
            for w in kept:
                if not _tick_sem(w.ant_name):
                    continue
                key = (eng, w.ant_name)
                seen_waits[key] = max(seen_waits.get(key, -1), w.wait_value)
            if len(kept) != len(si.on_wait):
                si.on_wait = kept
                ins.sync_info = si

    import bass_rust as _br

    for bb in nc.m.functions[0].blocks:
        while True:
            insns = list(bb.instructions)
            target = None
            for idx, ins in enumerate(insns):
                si = ins.sync_info
                if si is None:
                    continue
                if len(si.on_wait) > 1:
                    target = (idx, ins)
                    break
            if target is None:
                break
            idx, ins = target
            si = ins.sync_info
            waits = list(si.on_wait)
            if type(ins).__name__ == "InstDrain":
                room = max(0, 1 - len(si.on_update))
            else:
                room = 1
            keep, extra = waits[len(waits) - room :], waits[: len(waits) - room]
            pos = idx
            for i, w in enumerate(extra):
                nd = mybir.InstDrain(
                    name=f"{ins.name}-presync{i}", ins=[], outs=[],
                    bass_is_fusable=False,
                )
                nd.engine = ins.engine
                nd.sync_info = _br.SyncInfo(on_wait=[w], on_update=[])
                nc.register_instruction(nd, overwrite=True)
                bb.instructions.insert(pos, nd)
                pos += 1
            si.on_wait = keep
            ins.sync_info = si
    return nc


_NC_CACHE = {}


def _get_nc():
    if "nc" not in _NC_CACHE:
        _NC_CACHE["nc"] = _build()
    return _NC_CACHE["nc"]


def kernel(logits, labels):
    global LAST_RESULTS
    logits = np.ascontiguousarray(np.asarray(logits), dtype=np.float32)
    labels_i = np.asarray(labels).astype(np.int64)
    assert logits.shape == (N, C), logits.shape

    # host-side gather of the label logit (1% of input bytes)
    gvals = logits[np.arange(N), labels_i].astype(np.float32)

    # bias constants: -c'_b for the conf relu/sign passes, then -(c'_b - 2)
    cnrow = np.array(
        [-np.float32(t) for t in THRP] + [-np.float32(t) for t in THRU],
        dtype=np.float32,
    )
    cnarr = np.ascontiguousarray(np.broadcast_to(cnrow, (P, 32)))

    in_maps = []
    for c in range(NCORES):
        lo, hi = c * ROWS, (c + 1) * ROWS
        if hi <= N:
            xs = logits[lo:hi]
            gc = gvals[lo:hi]
        else:
            xs = np.concatenate(
                [logits[lo:], np.zeros((hi - N, C), np.float32)], axis=0
            )
            gc = np.concatenate([gvals[lo:], np.zeros(hi - N, np.float32)])
        in_maps.append(
            {
                "x": np.ascontiguousarray(xs.reshape(P, SPP * C)),
                "g": np.ascontiguousarray(gc.reshape(P, SPP)),
                "cn": cnarr,
            }
        )

    trace = bool(int(os.environ.get("ECE_TRACE", "0")))
    res = run_bass_kernel_spmd(
        _get_nc(), in_maps, core_ids=list(range(NCORES)), trace=trace
    )
    LAST_RESULTS = res

    R = np.zeros(NBINS, np.float64)       # sum relu(conf - c'_b)
    sgc = np.zeros(NBINS, np.float64)     # sum sign(conf - c'_b)
    sgu = np.zeros(NBINS, np.float64)     # sum sign(u - (c'_b - 2))
    sumconf = 0.0
    for out in res.results:
        ob = out["bins"].astype(np.float64)
        for ci in range(len(CHUNKS)):
            base = 49 * ci
            R += ob[:, base : base + 16].sum(axis=0)
            sgc += ob[:, base + 16 : base + 32].sum(axis=0)
            sgu += ob[:, base + 32 : base + 48].sum(axis=0)
            sumconf += ob[:, base + 48].sum()

    nn_ = (NTOT - sgc) / 2.0              # {conf <= c_b}, exact counts
    A = (NTOT - sgu) / 2.0                # {acc=1 and conf <= c_b}
    thrp64 = np.array([np.float64(t) for t in THRP])
    # S_b = sum of conf over {conf <= c_b}  (cumulative, includes pads)
    S = sumconf - R - thrp64 * (NTOT - nn_)
    # remove the PAD rows (conf_bf = bf16(0.01) <= every threshold, acc=1)
    S -= PAD * PAD_CONF_BF
    A -= PAD
    conf_sum = np.diff(S, prepend=0.0)
    acc_sum = np.diff(A, prepend=0.0)
    ece = np.abs(conf_sum - acc_sum).sum() / N
    return np.array([ece], dtype=np.float32)


# revision 28
# speedup vs baseline: 1.3101x; 1.0499x over previous
"""ECE loss kernel for Trainium2 (Bass/Tile), data-parallel over 8 NeuronCores.

Math (per sample row of logits[N, C]):
  conf = max softmax(x) = exp(max(x)) / sum(exp(x))
  acc  = (argmax(x) == label)
  ece  = sum_b |conf_sum[b] - acc_sum[b]| / N     (15 bins + empty tail)

Device pipeline per core (125184 rows as [128 partitions x 978 samples],
tiles of 16 samples x 100 classes):
  - DMA x tiles on the two HWDGE rings (sync/scalar)
  - ACT: E = exp(x) in place
  - Pool (GpSimd): two max tree-fold levels over E (100 -> 50 -> 25) —
    the otherwise-idle engine absorbs 3/4 of the row-max work; max-folds
    are numerically exact in any order
  - DVE: reduce_max over the 25-wide folds + reduce_sum over E (the only
    irreducible 1x pass), then conf/acc and 49 tensor_scalar+accum passes
    for the per-bin cumulative sums (wt/nn/av at 2x fp32) + total accuracy
Host: gathers g = x[i,label_i] (1% of bytes), pads 1472 zero rows, sums the
per-core [128, 49] outputs and finishes the tiny 16-bin reduction.
"""

import os

import numpy as np

import concourse.bass as bass
import concourse.mybir as mybir
import concourse.tile as tile
from concourse.bass_utils import run_bass_kernel_spmd

F32 = mybir.dt.float32
ALU = mybir.AluOpType
AX = mybir.AxisListType
ACTF = mybir.ActivationFunctionType

N = 1_000_000
C = 100
NCORES = 8
P = 128
SPP = 978                   # samples per partition (padded)
ROWS = P * SPP              # 125184 rows per core
NTOT = NCORES * ROWS        # 1001472
PAD = NTOT - N              # 1472 zero pad rows (conf=0.01, acc=1)
K = 32                      # samples per tile
SIZES = [K] * 30 + [18]     # sum = 978
CHUNKS = [(0, 512), (512, 896), (896, 978)]  # binning chunks (tile-aligned)
N_ACT_CHUNKS = 2            # first chunks bin on ACT (overlap the main loop);
                            # the small tail chunk bins on the then-idle DVE
NBINS = 16

LAST_RESULTS = None


def _bin_thresholds():
    """C_b = largest f32 y such that f32(15*y) <= b+1, for b = 0..14."""
    thr = []
    for b in range(15):
        tgt = np.float32(b + 1)

        def f(v):
            return np.float32(np.float32(15.0) * v)

        y = np.float32((b + 1) / 15.0)
        if f(y) <= tgt:
            while True:
                y2 = np.nextafter(y, np.float32(np.inf))
                if f(y2) <= tgt:
                    y = y2
                else:
                    break
        else:
            while f(y) > tgt:
                y = np.nextafter(y, np.float32(-np.inf))
        thr.append(np.float32(y))
    thr.append(np.float32(1e9))  # catch-all last segment
    return thr


THR = _bin_thresholds()


def _to_bf16(x):
    """Round f32 -> nearest-even bf16, returned as exactly-representable f32."""
    u = int(np.float32(x).view(np.uint32))
    upper, lower = u >> 16, u & 0xFFFF
    if lower > 0x8000 or (lower == 0x8000 and (upper & 1)):
        upper += 1
    return np.uint32(upper << 16).view(np.float32)


# bf16-representable bin boundaries: the device bins the bf16-rounded conf
# against these, and the host S_b identity uses the same exact values
THRB = [_to_bf16(t) for t in THR[:15]] + [np.float32(2.0 ** 30)]
PAD_CONF_BF = float(_to_bf16(np.float32(np.float32(1.0) * np.float32(1.0 / np.float32(100.0)))))


def _next_bf16_up(x):
    u = int(np.float32(x).view(np.uint32))
    return np.uint32(((u >> 16) + 1) << 16).view(np.float32)


# off-grid thresholds strictly between consecutive bf16 values: a bf16 conf
# can never equal one, so sign(conf - c') is exactly +-1 and
# (N - sum sign)/2 counts {conf <= c_b} exactly
THRP = [
    np.float32(
        np.float64(THRB[b]) + (np.float64(_next_bf16_up(THRB[b])) - np.float64(THRB[b])) / 2
    )
    for b in range(15)
] + [np.float32(1.5 * 2.0 ** 30)]
# u = conf - 2*acc thresholds: {u <= c'-2} == {acc=1 and conf <= c_b}
THRU = [np.float32(np.float32(t) - np.float32(2.0)) for t in THRP[:15]] + [
    np.float32(-0.5)
]


def _build():
    nc = bass.Bass(trn_type="TRN2")
    x = nc.dram_tensor("x", [P, SPP * C], F32, kind="ExternalInput")
    g = nc.dram_tensor("g", [P, SPP], F32, kind="ExternalInput")
    cn = nc.dram_tensor("cn", [P, 32], F32, kind="ExternalInput")
    bins = nc.dram_tensor("bins", [P, 49 * len(CHUNKS)], F32, kind="ExternalOutput")

    with tile.TileContext(nc) as tc:
        BF16 = mybir.dt.bfloat16
        with (
            tc.tile_pool(name="xin", bufs=6) as xin,
            tc.tile_pool(name="persist", bufs=1) as persist,
        ):
            g_sb = persist.tile([P, SPP], F32)
            em = persist.tile([P, SPP], F32)
            s_t = persist.tile([P, SPP], F32)
            rs = persist.tile([P, SPP], F32)
            conf_bf = persist.tile([P, SPP], BF16)
            acc_bf = persist.tile([P, SPP], BF16)
            junk_bf = persist.tile([P, SPP], BF16)
            bins_sb = persist.tile([P, 49 * len(CHUNKS)], F32)
            cn_sb = persist.tile([P, 32], F32)
            nc.gpsimd.dma_start(out=g_sb[:, :], in_=g[:, :])
            nc.gpsimd.dma_start(out=cn_sb[:, :], in_=cn[:, :])

            def emit_chunk(ci, lo, hi):
                sl = slice(lo, hi)
                base = 49 * ci
                nc.vector.reciprocal(rs[:, sl], s_t[:, sl])
                # conf_bf = bf16(em / s): all three per-bin sums see the SAME
                # rounded values, so per-sample bin assignment is consistent;
                # rounding only nudges boundary samples between adjacent bins
                nc.vector.tensor_mul(conf_bf[:, sl], em[:, sl], rs[:, sl])
                nc.scalar.activation(g_sb[:, sl], g_sb[:, sl], ACTF.Exp)
                nc.vector.tensor_tensor(
                    acc_bf[:, sl], g_sb[:, sl], em[:, sl], op=ALU.is_equal
                )
                if ci < N_ACT_CHUNKS:
                    # u = conf - 2*acc  (reuses rs; exact in fp32)
                    nc.vector.scalar_tensor_tensor(
                        rs[:, sl], acc_bf[:, sl], -2.0, conf_bf[:, sl],
                        op0=ALU.mult, op1=ALU.add,
                    )
                junk = s_t  # dead after reciprocal
                if ci < N_ACT_CHUNKS:
                    nc.scalar.activation(
                        junk[:, sl], conf_bf[:, sl], ACTF.Copy,
                        accum_out=bins_sb[:, base + 48 : base + 49],
                    )
                    for b in range(NBINS):
                        nc.scalar.activation(
                            junk[:, sl], conf_bf[:, sl], ACTF.Relu,
                            bias=cn_sb[:, b : b + 1],
                            accum_out=bins_sb[:, base + b : base + b + 1],
                        )
                    for b in range(NBINS):
                        nc.scalar.activation(
                            junk[:, sl], conf_bf[:, sl], ACTF.Sign,
                            bias=cn_sb[:, b : b + 1],
                            accum_out=bins_sb[:, base + 16 + b : base + 17 + b],
                        )
                    for b in range(NBINS):
                        nc.scalar.activation(
                            junk[:, sl], rs[:, sl], ACTF.Sign,
                            bias=cn_sb[:, 16 + b : 17 + b],
                            accum_out=bins_sb[:, base + 32 + b : base + 33 + b],
                        )
                else:
                    # tail chunk: DVE min/count binning (DVE is idle here)
                    for b in range(NBINS):
                        cb = float(THRB[b])
                        nc.vector.tensor_scalar(
                            junk_bf[:, sl], conf_bf[:, sl], cb, None,
                            op0=ALU.min, op1=ALU.add,
                            accum_out=bins_sb[:, base + b : base + b + 1],
                        )
                        nc.vector.tensor_scalar(
                            junk_bf[:, sl], conf_bf[:, sl], cb, None,
                            op0=ALU.is_le, op1=ALU.add,
                            accum_out=bins_sb[:, base + 16 + b : base + 17 + b],
                        )
                        nc.vector.scalar_tensor_tensor(
                            junk_bf[:, sl], conf_bf[:, sl], cb, acc_bf[:, sl],
                            op0=ALU.is_le, op1=ALU.mult,
                            accum_out=bins_sb[:, base + 32 + b : base + 33 + b],
                        )

            dma_engines = [nc.sync, nc.scalar]
            off = 0
            ci = 0
            for t, k in enumerate(SIZES):
                kc = k * C
                xt = xin.tile([P, K * C], F32, tag="xt")
                dma_engines[t % 2].dma_start(
                    out=xt[:, :kc], in_=x[:, off * C : (off + k) * C]
                )
                nc.scalar.activation(xt[:, :kc], xt[:, :kc], ACTF.Exp)
                ev = xt[:, :kc].rearrange("p (k c) -> p k c", c=C)
                nc.vector.reduce_max(
                    out=em[:, off : off + k], in_=ev[:, :, :], axis=AX.X
                )
                nc.vector.reduce_sum(
                    out=s_t[:, off : off + k], in_=ev[:, :, :], axis=AX.X
                )
                off += k
                if ci < len(CHUNKS) and CHUNKS[ci][1] == off:
                    emit_chunk(ci, CHUNKS[ci][0], CHUNKS[ci][1])
                    ci += 1
            assert ci == len(CHUNKS)
            nc.sync.dma_start(out=bins[:, :], in_=bins_sb[:, :])

    # ---- sync-command budget fixes (instructions carry <= 2 sync commands:
    # completion update + at most one wait).  Drop waits provably covered by
    # earlier waits on the same engine, then split any remaining multi-wait
    # instruction across preceding same-engine drains.
    import re as _re

    def _tick_sem(name):
        return bool(_re.match(
            r"^(Activation|DVE|PE|Pool|SP|DMAHW\d+|DMASW\d+)_\d+$", name
        ))

    seen_waits = {}
    for bb in nc.m.functions[0].blocks:
        for ins in bb.instructions:
            si = ins.sync_info
            if si is None:
                continue
            tname = type(ins).__name__
            if tname == "InstEventSemaphore":
                continue
            eng = str(ins.engine).split(".")[-1]
            kept = list(si.on_wait)
            if tname not in ("InstDMACopy", "InstDrain") and len(kept) > 1:
                # same-engine waits are redundant (program order)
                kept = [w for w in kept if not w.ant_name.startswith(f"{eng}_")]
            kept2 = []
            for w in kept:
                if not _tick_sem(w.ant_name):
                    kept2.append(w)
                elif seen_waits.get((eng, w.ant_name), -1) < w.wait_value:
                    kept2.append(w)
            kept = kept2
            for w in kept:
                if not _tick_sem(w.ant_name):
                    continue
                key = (eng, w.ant_name)
                seen_waits[key] = max(seen_waits.get(key, -1), w.wait_value)
            if len(kept) != len(si.on_wait):
                si.on_wait = kept
                ins.sync_info = si

    import bass_rust as _br

    for bb in nc.m.functions[0].blocks:
        while True:
            insns = list(bb.instructions)
            target = None
            for idx, ins in enumerate(insns):
                si = ins.sync_info
                if si is None:
                    continue
                if len(si.on_wait) > 1:
                    target = (idx, ins)
                    break
            if target is None:
                break
            idx, ins = target
            si = ins.sync_info
            waits = list(si.on_wait)
            if type(ins).__name__ == "InstDrain":
                room = max(0, 1 - len(si.on_update))
            else:
                room = 1
            keep, extra = waits[len(waits) - room :], waits[: len(waits) - room]
            pos = idx
            for i, w in enumerate(extra):
                nd = mybir.InstDrain(
                    name=f"{ins.name}-presync{i}", ins=[], outs=[],
                    bass_is_fusable=False,
                )
                nd.engine = ins.engine
                nd.sync_info = _br.SyncInfo(on_wait=[w], on_update=[])
                nc.register_instruction(nd, overwrite=True)
                bb.instructions.insert(pos, nd)
                pos += 1
            si.on_wait = keep
            ins.sync_info = si
    return nc


_NC_CACHE = {}


def _get_nc():
    if "nc" not in _NC_CACHE:
        _NC_CACHE["nc"] = _build()
    return _NC_CACHE["nc"]


def kernel(logits, labels):
    global LAST_RESULTS
    logits = np.ascontiguousarray(np.asarray(logits), dtype=np.float32)
    labels_i = np.asarray(labels).astype(np.int64)
    assert logits.shape == (N, C), logits.shape

    # host-side gather of the label logit (1% of input bytes)
    gvals = logits[np.arange(N), labels_i].astype(np.float32)

    # bias constants: -c'_b for the conf relu/sign passes, then -(c'_b - 2)
    cnrow = np.array(
        [-np.float32(t) for t in THRP] + [-np.float32(t) for t in THRU],
        dtype=np.float32,
    )
    cnarr = np.ascontiguousarray(np.broadcast_to(cnrow, (P, 32)))

    in_maps = []
    for c in range(NCORES):
        lo, hi = c * ROWS, (c + 1) * ROWS
        if hi <= N:
            xs = logits[lo:hi]
            gc = gvals[lo:hi]
        else:
            xs = np.concatenate(
                [logits[lo:], np.zeros((hi - N, C), np.float32)], axis=0
            )
            gc = np.concatenate([gvals[lo:], np.zeros(hi - N, np.float32)])
        in_maps.append(
            {
                "x": np.ascontiguousarray(xs.reshape(P, SPP * C)),
                "g": np.ascontiguousarray(gc.reshape(P, SPP)),
                "cn": cnarr,
            }
        )

    trace = bool(int(os.environ.get("ECE_TRACE", "0")))
    res = run_bass_kernel_spmd(
        _get_nc(), in_maps, core_ids=list(range(NCORES)), trace=trace
    )
    LAST_RESULTS = res

    R = np.zeros(NBINS, np.float64)       # sum relu(conf - c'_b)   [ACT chunks]
    sgc = np.zeros(NBINS, np.float64)     # sum sign(conf - c'_b)
    sgu = np.zeros(NBINS, np.float64)     # sum sign(u - (c'_b - 2))
    sumconf = 0.0
    wt = np.zeros(NBINS, np.float64)      # sum min(conf, c_b)      [DVE chunks]
    nn_d = np.zeros(NBINS, np.float64)    # {conf <= c_b}
    av_d = np.zeros(NBINS, np.float64)    # {acc=1 and conf <= c_b}
    for out in res.results:
        ob = out["bins"].astype(np.float64)
        for ci in range(len(CHUNKS)):
            base = 49 * ci
            if ci < N_ACT_CHUNKS:
                R += ob[:, base : base + 16].sum(axis=0)
                sgc += ob[:, base + 16 : base + 32].sum(axis=0)
                sgu += ob[:, base + 32 : base + 48].sum(axis=0)
                sumconf += ob[:, base + 48].sum()
            else:
                wt += ob[:, base : base + 16].sum(axis=0)
                nn_d += ob[:, base + 16 : base + 32].sum(axis=0)
                av_d += ob[:, base + 32 : base + 48].sum(axis=0)

    # per-scheme sample totals (positional; pads included)
    L_act = sum(hi - lo for ci, (lo, hi) in enumerate(CHUNKS) if ci < N_ACT_CHUNKS)
    L_dve = sum(hi - lo for ci, (lo, hi) in enumerate(CHUNKS) if ci >= N_ACT_CHUNKS)
    L_act *= P * NCORES
    L_dve *= P * NCORES

    nn_a = (L_act - sgc) / 2.0            # {conf <= c_b}, exact counts
    A_a = (L_act - sgu) / 2.0             # {acc=1 and conf <= c_b}
    thrp64 = np.array([np.float64(t) for t in THRP])
    S_a = sumconf - R - thrp64 * (L_act - nn_a)

    thrb64 = np.array([np.float64(t) for t in THRB])
    S_d = wt - thrb64 * (L_dve - nn_d)

    # cumulative totals; then remove the PAD rows (conf_bf = bf16(0.01) <=
    # every threshold, acc=1)
    S = S_a + S_d - PAD * PAD_CONF_BF
    A = A_a + av_d - PAD
    conf_sum = np.diff(S, prepend=0.0)
    acc_sum = np.diff(A, prepend=0.0)
    ece = np.abs(conf_sum - acc_sum).sum() / N
    return np.array([ece], dtype=np.float32)
